# revision 4
# baseline (speedup 1.0000x reference)
"""GQA attention block (B=2, S=2048, D=2048, H=32, KVH=8, HD=64, RoPE) on 8
Trainium2 NeuronCores — v2.

Sharding as v1: core = (batch, kv-head pair); host sums 4 wo-partials/batch.

v2 changes vs v1:
- fp16 everywhere on-chip (x, weights, k/q/v', probs, attn, wo, output):
  halves DMA and SBUF, enables DVE 2x modes and FWL weight loads. Numerics:
  fp16 inputs measured 3.3e-4 absmax-rel vs fp32 reference (tolerance 2e-2).
- All x chunk DMAs prefetched up front (fp16 x fits SBUF whole); single flat
  pool scope per rep.
- Minimal preamble: K0/V0'/Q-chunk0 only; attention starts ~15us in.
- qc0 iterates key-block-outer (kcB, p, kc) with PV accumulated into SBUF
  acc tiles between blocks, so early attention consumes keys no faster than
  the x DMA stream delivers them; qc1-3 keep pair-outer PSUM accumulation.
- K/V'/Q chunk 1-3 projections and all wo chunks form one global
  deadline-ordered filler stream popped between attention iterations
  (qc0 nf=3, qc1/qc2 nf=2, qc3 nf=1, remainder in tail).
- ACT exp table preloaded at t~0 via a dummy exp so the first real exp
  doesn't pay the ~2.7us table load.
- qc1-3 issue scores in adjacent kc-pairs (second pair's row-tiled
  LDWEIGHTS hide inside the first pair's streams), PV lags one kc-pair;
  boundary DVE ops (PV staging, norm) are issued before the deferred
  filler pops so the pv PSUM bank frees without stalling the PE FIFO.
"""

import sys

import numpy as np

if "/opt/trn_rl_repo" not in sys.path:
    sys.path.insert(0, "/opt/trn_rl_repo")

B, S, D = 2, 2048, 2048
H, KVH = 32, 8
HD = D // H            # 64
NREP = H // KVH        # 4
ROPE_THETA = 10000.0
N_CORES = 8
P = 128
NQ = 512               # q rows per core (8 heads * 64)
NKV = 128              # k/v rows per core (2 kv heads * 64)
KO = D // P            # 16 contraction chunks for projections
SC = S // 512          # 4 column chunks of 512
KH = 2                 # x streams in [P, KH, 512] tiles
NKQ = KO // KH         # 8 quarter-tiles per column chunk


def _rope_tables():
    """cos/sin tables [P, S]; partition layout per 64-row head block:
    rows 0:32 = even dims ("a"), rows 32:64 = odd dims ("b").
    a' = a*cos - b*sin ; b' = b*cos + a*sin; the swap-multiply in0 reads the
    OTHER block, so sin carries -sin on a-rows, +sin on b-rows."""
    freqs = (1.0 / (ROPE_THETA **
                    (np.arange(0, HD, 2, dtype=np.float32) / np.float32(HD))))
    freqs = freqs.astype(np.float32)                                  # [32]
    ang = (np.arange(S, dtype=np.float32)[None, :] * freqs[:, None])  # [32,S]
    cos = np.cos(ang).astype(np.float32)
    sin = np.sin(ang).astype(np.float32)
    ctab = np.concatenate([cos, cos, cos, cos], axis=0)               # [128,S]
    stab = np.concatenate([-sin, sin, -sin, sin], axis=0)             # [128,S]
    return ctab, stab


def _build_bass(reps: int = 1, wo_paired: bool = False,
                sc_paired: bool = True):
    import concourse.bass as bass  # noqa: F401
    import concourse.tile as tile
    from concourse import bacc, mybir
    from concourse.masks import make_identity

    F32 = mybir.dt.float32
    F16 = mybir.dt.float16
    EXP = mybir.ActivationFunctionType.Exp
    COPY = mybir.ActivationFunctionType.Copy
    MULT = mybir.AluOpType.mult
    ADD = mybir.AluOpType.add

    nc = bacc.Bacc("TRN2", target_bir_lowering=False, debug=False,
                   num_devices=N_CORES)

    # x/wq/wk/wv are host-packed into per-partition contiguous streams so
    # every DMA descriptor is a single long run (>=2KB); the rearranged-AP
    # form decomposes into 256B descriptors and runs ~4x slower.
    xP = nc.dram_tensor("xP", [P, SC * KO * 512], F16, kind="ExternalInput")
    wqP = nc.dram_tensor("wqP", [P, 4 * KO * P], F16, kind="ExternalInput")
    wkP = nc.dram_tensor("wkP", [P, KO * P], F16, kind="ExternalInput")
    wvP = nc.dram_tensor("wvP", [P, KO * P], F16, kind="ExternalInput")
    woT = nc.dram_tensor("woT", [NQ, D], F16, kind="ExternalInput")
    ck = nc.dram_tensor("ck", [P, S], F16, kind="ExternalInput")
    sk = nc.dram_tensor("sk", [P, S], F16, kind="ExternalInput")
    maskT = nc.dram_tensor("maskT", [P, KO], F32, kind="ExternalInput")
    part = nc.dram_tensor("part", [S, D], F16, kind="ExternalOutput")

    woT_r = woT.ap().rearrange("(dk p) e -> p dk e", p=P)   # [128, 4, 2048]

    with tile.TileContext(nc) as tc:
      for rep in range(reps):
        with tc.tile_pool(name="persist", bufs=1) as persist, \
             tc.tile_pool(name="acc", bufs=8) as accpool, \
             tc.tile_pool(name="probs", bufs=4) as prpool, \
             tc.tile_pool(name="nrm", bufs=2) as nrmpool, \
             tc.tile_pool(name="sw", bufs=2) as swpool, \
             tc.tile_pool(name="xp", bufs=32) as xpool, \
             tc.tile_pool(name="wp", bufs=1) as wpool, \
             tc.tile_pool(name="oev", bufs=4) as oevpool, \
             tc.tile_pool(name="mmPS", bufs=2, space="PSUM") as mmps, \
             tc.tile_pool(name="attnPS", bufs=2, space="PSUM") as spool, \
             tc.tile_pool(name="pvPS", bufs=2, space="PSUM") as pvpool:

            qsb = [persist.tile([P, S], F16, tag=f"qsb{m}",
                                name=f"qsb{m}_{rep}")
                   for m in range(4)]
            kab = persist.tile([P, S], F16, tag="kab")
            # V' layout: cols 0:64 = V dims, col HD = mask (softmax z source)
            vpr = [persist.tile([P, KO, HD + 1], F16, tag=f"vpr{i}",
                                name=f"vpr{i}_{rep}")
                   for i in range(2)]
            msk = persist.tile([P, KO], F32, tag="msk")
            ident = persist.tile([P, P], F32, tag="ident")
            tab_ck = persist.tile([P, S], F16, tag="tab_ck")
            tab_sk = persist.tile([P, S], F16, tag="tab_sk")
            attn = qsb

            # ---- DMA issue order == deadline order ----
            wk_sb = wpool.tile([P, KO * NKV], F16, tag="wk")
            nc.sync.dma_start(wk_sb[:], wkP.ap())
            make_identity(nc, ident[:])
            # dummy exp: pulls the ACT exp table load to t~0 (reads ident,
            # which needs no DMA, so it issues immediately)
            scratch = swpool.tile([P, 4], F32, tag="scr")
            nc.scalar.activation(scratch[:], ident[:, 0:4], EXP)
            nc.sync.dma_start(msk[:], maskT.ap())

            def make_xq(c):
                out = []
                for kq in range(NKQ):
                    xq = xpool.tile([P, KH * 512], F16, tag="xq",
                                    name=f"xq{c}_{kq}_{rep}")
                    off = (c * NKQ + kq) * KH * 512
                    nc.sync.dma_start(xq[:], xP.ap()[:, off:off + KH * 512])
                    out.append(xq)
                return out

            xqs = [None] * SC
            xqs[0] = make_xq(0)
            # rope tables and wq split by chunk: only cols 0:512 / m=0 gate
            # the preamble; the rest streams behind x.
            nc.sync.dma_start(tab_ck[:, 0:512], ck.ap()[:, 0:512])
            nc.sync.dma_start(tab_sk[:, 0:512], sk.ap()[:, 0:512])
            wq_sb = wpool.tile([P, 4 * KO * P], F16, tag="wq")
            nc.sync.dma_start(wq_sb[:, 0:KO * P], wqP.ap()[:, 0:KO * P])
            wv_sb = wpool.tile([P, KO * NKV], F16, tag="wv")
            nc.sync.dma_start(wv_sb[:], wvP.ap())
            for m in range(1, 4):
                nc.sync.dma_start(wq_sb[:, m * KO * P:(m + 1) * KO * P],
                                  wqP.ap()[:, m * KO * P:(m + 1) * KO * P])
            nc.sync.dma_start(tab_ck[:, 512:S], ck.ap()[:, 512:S])
            nc.sync.dma_start(tab_sk[:, 512:S], sk.ap()[:, 512:S])
            for c in range(1, SC):
                xqs[c] = make_xq(c)
            wot_sb = wpool.tile([P, 4, S], F16, tag="wot")
            nc.sync.dma_start(wot_sb[:], woT_r)

            # mask column of V' (independent of V values)
            for i in range(2):
                nc.vector.tensor_copy(vpr[i][:, :, HD], msk[:])

            def rope_evac(ps, dst_tile, s0, scale):
                """dst[:, s0:s0+512] = rope(ps * scale), tables at cols s0."""
                dst = dst_tile[:, s0:s0 + 512]
                c_sl = tab_ck[:, s0:s0 + 512]
                s_sl = tab_sk[:, s0:s0 + 512]
                sw = swpool.tile([P, 512], F16, tag="sw")
                for o in range(0, P, 64):
                    nc.vector.scalar_tensor_tensor(
                        sw[o:o + 32, :], ps[o + 32:o + 64, :], scale,
                        s_sl[o:o + 32, :], MULT, MULT)
                    nc.vector.scalar_tensor_tensor(
                        sw[o + 32:o + 64, :], ps[o:o + 32, :], scale,
                        s_sl[o + 32:o + 64, :], MULT, MULT)
                nc.vector.scalar_tensor_tensor(
                    dst, ps[:], scale, c_sl, MULT, MULT)
                nc.vector.tensor_tensor(dst, dst, sw[:], ADD)

            def xsl(c, k):
                return xqs[c][k // KH][:, (k % KH) * 512:(k % KH + 1) * 512]

            def proj_mm(ps, c, wsl):
                for k in range(KO):
                    nc.tensor.matmul(
                        ps[:], wsl(k), xsl(c, k),
                        start=(k == 0), stop=(k == KO - 1))

            def wk_sl(k):
                return wk_sb[:, k * NKV:(k + 1) * NKV]

            def wv_sl(k):
                return wv_sb[:, k * NKV:(k + 1) * NKV]

            def wq_sl(m, k):
                return wq_sb[:, (m * KO + k) * P:(m * KO + k + 1) * P]

            def vprime(c, vsb):
                """vpr[:, 4c:4c+4, 0:HD] from vsb [128=2*HD, 512 keys]."""
                for i in range(2):
                    for kq in range(4):
                        kc = c * 4 + kq
                        pst = mmps.tile([P, 512], F32, tag="mm",
                                        name=f"pst{c}_{i}{kq}_{rep}"
                                        )[:, 0:HD]
                        nc.tensor.transpose(
                            pst[:],
                            vsb[i * HD:(i + 1) * HD,
                                kq * P:(kq + 1) * P],
                            ident[i * HD:(i + 1) * HD,
                                  i * HD:(i + 1) * HD])
                        nc.scalar.activation(
                            vpr[i][:, kc, 0:HD], pst[:], COPY,
                            scale=msk[:, kc:kc + 1])

            # ---- preamble: K0, V0', Q chunk 0 (serial) ----
            ps = mmps.tile([P, 512], F32, tag="mm", name=f"pk0_{rep}")
            proj_mm(ps, 0, wk_sl)
            rope_evac(ps, kab, 0, 1.0)
            ps = mmps.tile([P, 512], F32, tag="mm", name=f"pv0_{rep}")
            proj_mm(ps, 0, wv_sl)
            vsb = swpool.tile([P, 512], F32, tag="vsb")
            nc.scalar.activation(vsb[:], ps[:], COPY)
            for m in range(4):
                ps = mmps.tile([P, 512], F32, tag="mm", name=f"pq0{m}_{rep}")
                proj_mm(ps, 0, lambda k, m=m: wq_sl(m, k))
                rope_evac(ps, qsb[m], 0, 0.125)
            vprime(0, vsb)  # last: vpr0 first needed by PV at attention it.1

            # ---- filler steps (single global deadline-ordered stream) ----
            def kv_steps(c):
                """Chunk c: 16 K + 16 V + 8 V' steps (one PE op each)."""
                box = {}
                for k in range(KO):
                    def f(k=k):
                        if k == 0:
                            box["ps"] = mmps.tile([P, 512], F32, tag="mm",
                                                  name=f"fk{c}_{rep}")
                        nc.tensor.matmul(
                            box["ps"][:], wk_sl(k), xsl(c, k),
                            start=(k == 0), stop=(k == KO - 1))
                        if k == KO - 1:
                            rope_evac(box["ps"], kab, c * 512, 1.0)
                    yield f
                for k in range(KO):
                    def f(k=k):
                        if k == 0:
                            box["psv"] = mmps.tile([P, 512], F32, tag="mm",
                                                   name=f"fv{c}_{rep}")
                        nc.tensor.matmul(
                            box["psv"][:], wv_sl(k), xsl(c, k),
                            start=(k == 0), stop=(k == KO - 1))
                        if k == KO - 1:
                            box["vsb"] = swpool.tile([P, 512], F32,
                                                     tag="vsb",
                                                     name=f"fvs{c}_{rep}")
                            nc.scalar.activation(box["vsb"][:],
                                                 box["psv"][:], COPY)
                    yield f
                for i in range(2):
                    for kq in range(4):
                        def f(i=i, kq=kq):
                            kc = c * 4 + kq
                            pst = mmps.tile([P, 512], F32, tag="mm",
                                            name=f"fp{c}_{i}{kq}_{rep}"
                                            )[:, 0:HD]
                            nc.tensor.transpose(
                                pst[:],
                                box["vsb"][i * HD:(i + 1) * HD,
                                           kq * P:(kq + 1) * P],
                                ident[i * HD:(i + 1) * HD,
                                      i * HD:(i + 1) * HD])
                            nc.scalar.activation(
                                vpr[i][:, kc, 0:HD], pst[:], COPY,
                                scale=msk[:, kc:kc + 1])
                        yield f

            def q_steps(c):
                box = {}
                for m in range(4):
                    for k in range(KO):
                        def f(m=m, k=k):
                            if k == 0:
                                box["ps"] = mmps.tile(
                                    [P, 512], F32, tag="mm",
                                    name=f"fq{c}_{m}_{rep}")
                            nc.tensor.matmul(
                                box["ps"][:], wq_sl(m, k), xsl(c, k),
                                start=(k == 0), stop=(k == KO - 1))
                            if k == KO - 1:
                                rope_evac(box["ps"], qsb[m], c * 512, 0.125)
                        yield f

            def _wo_evac(po, qt, n, on_act):
                ot = oevpool.tile([P, 512], F16, tag="ot")
                if on_act:
                    nc.scalar.activation(ot[:], po[:], COPY)
                else:
                    nc.vector.tensor_copy(ot[:], po[:])
                nc.sync.dma_start(
                    part.ap()[qt * P:(qt + 1) * P,
                              n * 512:(n + 1) * 512], ot[:])

            def wo_steps(qt, on_act=False):
                box = {}
                if wo_paired:
                    # dk-outer with n-pairs: attn[dk] stationary reused by
                    # two consecutive MMs -> half the LDWEIGHTS switches
                    for half in range(2):
                        for dk in range(4):
                            for n in (2 * half, 2 * half + 1):
                                def f(dk=dk, n=n):
                                    if dk == 0:
                                        box[n] = mmps.tile(
                                            [P, 512], F32, tag="mm",
                                            name=f"fwo{qt}_{n}_{rep}")
                                    nc.tensor.matmul(
                                        box[n][:],
                                        attn[dk][:, qt * P:(qt + 1) * P],
                                        wot_sb[:, dk,
                                               n * 512:(n + 1) * 512],
                                        start=(dk == 0), stop=(dk == 3))
                                    if dk == 3:
                                        _wo_evac(box[n], qt, n, on_act)
                                yield f
                    return
                for n in range(SC):
                    for dk in range(4):
                        def f(n=n, dk=dk):
                            if dk == 0:
                                box["po"] = mmps.tile(
                                    [P, 512], F32, tag="mm",
                                    name=f"fwo{qt}_{n}_{rep}")
                            nc.tensor.matmul(
                                box["po"][:],
                                attn[dk][:, qt * P:(qt + 1) * P],
                                wot_sb[:, dk, n * 512:(n + 1) * 512],
                                start=(dk == 0), stop=(dk == 3))
                            if dk == 3:
                                _wo_evac(box["po"], qt, n, on_act)
                        yield f

            def chain(*gens):
                for g in gens:
                    yield from g

            # Phase-aligned filler streams: wo chunk qt reads attn columns
            # qt*128, final only after phase qt//4 — so qt0-7 run in qc2,
            # qt8-11 in qc3, qt12-15 in the tail.
            fillA = chain(kv_steps(1), kv_steps(2), kv_steps(3),
                          q_steps(1), q_steps(2), q_steps(3))   # 312 steps
            fillB = chain(*[wo_steps(qt) for qt in range(0, 8)])   # 128
            fillC = chain(*[wo_steps(qt) for qt in range(8, 12)])  # 64
            fill = fillA

            def pop(nf):
                for _ in range(nf):
                    step = next(fill, None)
                    if step is not None:
                        step()

            def attn_scores(p, kc, qlo):
                ss = spool.tile([P, 1024], F32, tag="ss")
                for i in range(2):
                    nc.tensor.matmul(
                        ss[:, i * 512:(i + 1) * 512],
                        kab[i * HD:(i + 1) * HD, kc * P:(kc + 1) * P],
                        qsb[p][i * HD:(i + 1) * HD, qlo:qlo + 512],
                        start=True, stop=True,
                        tile_position=(i * HD, 0))
                return ss

            def attn_exp(ss):
                pr = prpool.tile([P, 1024], F16, tag="pr")
                nc.scalar.activation(pr[:], ss[:], EXP)
                return pr

            def attn_sc_exp(p, kc, qlo):
                """Scores pair + exp for one kc; returns the probs tile."""
                return attn_exp(attn_scores(p, kc, qlo))

            def attn_pv(pr, kc, pvs, start, stop):
                for i in range(2):
                    nc.tensor.matmul(
                        pvs[i][0:HD + 1, :],
                        vpr[i][:, kc, :],
                        pr[:, i * 512:(i + 1) * 512],
                        start=start, stop=stop)

            def norm_sbuf(p, qc, src0, src1):
                """attn[p][:, qc*512:...] = src[0:HD] / src[HD] (per head)."""
                for i, src in enumerate((src0, src1)):
                    qb = i * HD
                    zrow = nrmpool.tile([1, 512], F32, tag="zrow")
                    nc.vector.tensor_copy(zrow[:], src[HD:HD + 1, :])
                    rz = nrmpool.tile([1, 512], F32, tag="rz")
                    nc.vector.reciprocal_approx_fast(rz[:], zrow[:])
                    rzb = nrmpool.tile([HD, 512], F32, tag="rzb")
                    nc.gpsimd.partition_broadcast(rzb[:], rz[:])
                    nc.vector.tensor_tensor(
                        attn[p][qb:qb + HD, qc * 512:(qc + 1) * 512],
                        src[0:HD, :], rzb[:], MULT)

            # ---- qc0: key-block-outer with SBUF PV accumulation ----
            accs = [[accpool.tile([HD + 1, 512], F32, tag="acc",
                                  name=f"acc{p}_{i}_{rep}")
                     for i in range(2)] for p in range(4)]
            for kcB in range(4):
                for p in range(4):
                    pvA = pvpool.tile([P, 512], F32, tag="pv",
                                      name=f"pv0A_{rep}")
                    pvB = pvpool.tile([P, 512], F32, tag="pv",
                                      name=f"pv0B_{rep}")
                    pvs = (pvA, pvB)
                    # PV lags scores/exp by one kc so a busy pv bank never
                    # head-of-line-blocks the PE queue at block boundaries
                    # last iteration's pops are deferred until after the
                    # acc adds: the boundary DVE ops must not queue behind
                    # filler rope work, or the pv bank frees late and
                    # stalls the next block's PV. Scores issued in adjacent
                    # kc-pairs (as in qc1-3), PV lags one pair.
                    pend = []
                    for jh in range(2):
                        k0 = kcB * 4 + 2 * jh
                        ss0 = attn_scores(p, k0, 0)
                        ss1 = attn_scores(p, k0 + 1, 0)
                        pr0 = attn_exp(ss0)
                        if pend:
                            attn_pv(pend[0], k0 - 2, pvs,
                                    start=True, stop=False)
                        pr1 = attn_exp(ss1)
                        if pend:
                            attn_pv(pend[1], k0 - 1, pvs,
                                    start=False, stop=False)
                        pend = [pr0, pr1]
                        pop(6 if jh == 0 else 3)
                    attn_pv(pend[0], kcB * 4 + 2, pvs,
                            start=False, stop=False)
                    attn_pv(pend[1], kcB * 4 + 3, pvs,
                            start=False, stop=True)
                    for i in range(2):
                        if kcB == 0:
                            nc.vector.tensor_copy(accs[p][i][:],
                                                  pvs[i][0:HD + 1, :])
                        else:
                            nc.vector.tensor_tensor(
                                accs[p][i][:], accs[p][i][:],
                                pvs[i][0:HD + 1, :], ADD)
                    if kcB == 3:
                        norm_sbuf(p, 0, accs[p][0], accs[p][1])
                    pop(3)

            # ---- qc1..qc3: pair-outer, PSUM accumulation ----
            def attention_qc(qc, nf):
                for p in range(4):
                    pvA = pvpool.tile([P, 512], F32, tag="pv",
                                      name=f"pvA_{rep}")
                    pvB = pvpool.tile([P, 512], F32, tag="pv",
                                      name=f"pvB_{rep}")
                    pvs = (pvA, pvB)
                    if sc_paired:
                        pend = []
                        for kch in range(KO // 2):
                            k0, k1 = 2 * kch, 2 * kch + 1
                            ss0 = attn_scores(p, k0, qc * 512)
                            ss1 = attn_scores(p, k1, qc * 512)
                            pr0 = attn_exp(ss0)
                            if pend:
                                attn_pv(pend[0], k0 - 2, pvs,
                                        start=(kch == 1), stop=False)
                            pr1 = attn_exp(ss1)
                            if pend:
                                attn_pv(pend[1], k0 - 1, pvs,
                                        start=False, stop=False)
                            pend = [pr0, pr1]
                            if kch < KO // 2 - 1:
                                pop(2 * nf)
                        attn_pv(pend[0], KO - 2, pvs,
                                start=False, stop=False)
                        attn_pv(pend[1], KO - 1, pvs,
                                start=False, stop=True)
                    else:
                        prev = None
                        for kc in range(KO):
                            pr = attn_sc_exp(p, kc, qc * 512)
                            if prev is not None:
                                attn_pv(prev, kc - 1, pvs,
                                        start=(kc == 1), stop=False)
                            prev = pr
                            if kc < KO - 1:
                                pop(nf)
                        attn_pv(prev, KO - 1, pvs, start=False, stop=True)
                    # stage PV to SBUF so the psum bank frees immediately;
                    # issued before the deferred pops so these DVE ops are
                    # not stuck behind filler rope work. For the very last
                    # pair the exp stream is over: stage via ACT so the
                    # norm chain (gating the wo tail) starts sooner.
                    last = (qc == 3 and p == 3)
                    sb0 = nrmpool.tile([HD + 1, 512], F32, tag="pvsb")
                    sb1 = nrmpool.tile([HD + 1, 512], F32, tag="pvsb")
                    if last:
                        nc.scalar.activation(sb0[:], pvs[0][0:HD + 1, :],
                                             COPY)
                        nc.scalar.activation(sb1[:], pvs[1][0:HD + 1, :],
                                             COPY)
                    else:
                        nc.vector.tensor_copy(sb0[:], pvs[0][0:HD + 1, :])
                        nc.vector.tensor_copy(sb1[:], pvs[1][0:HD + 1, :])
                    norm_sbuf(p, qc, sb0, sb1)
                    pop(2 * nf if sc_paired else nf)

            attention_qc(1, 2)
            for step in fill:  # drain any fillA remainder before qc2
                step()
            fill = fillB
            attention_qc(2, 2)
            for step in fill:
                step()
            fill = fillC
            attention_qc(3, 1)
            for step in fill:
                step()

            # ---- tail: remaining wo chunks ----
            for qt in range(12, KO):
                for step in wo_steps(qt, on_act=(qt % 2 == 0)):
                    step()

    nc.compile()
    return nc


_PERM = np.concatenate([np.arange(0, HD, 2), np.arange(1, HD, 2)])


def _prep_core_inputs(x, wq, wk, wv, wo, attention_mask, core, tables):
    b = core // 4
    g = core % 4
    ctab, stab = tables

    # head order [0,4,1,5,2,6,3,7]: tile m holds heads (m, m+4) so head h
    # sits at partition base (h//4)*64 == its kv head's base in kab
    hperm = np.array([0, 4, 1, 5, 2, 6, 3, 7])
    qrows = wq[8 * g * HD:(8 * g + 8) * HD]          # [512, 2048]
    qrows = qrows.reshape(8, HD, D)[hperm][:, _PERM, :].reshape(NQ, D)
    krows = wk[2 * g * HD:(2 * g + 2) * HD]          # [128, 2048]
    krows = krows.reshape(2, HD, D)[:, _PERM, :].reshape(NKV, D)
    vrows = wv[2 * g * HD:(2 * g + 2) * HD]          # [128, 2048]
    wocols = wo[:, 8 * g * HD:(8 * g + 8) * HD]      # [2048, 512]
    wocols = wocols.reshape(D, 8, HD)[:, hperm, :].reshape(D, NQ)

    maskf = attention_mask[b].astype(np.float32)     # [S]
    maskT = np.ascontiguousarray(maskf.reshape(KO, P).T)   # [128, 16]

    f16 = np.float16
    # pack x/wq/wk/wv into per-partition contiguous DMA streams:
    # xP[p, (c, kq, j, s)] = xT[(kq*KH+j)*128+p, c*512+s]
    xT16 = x[b].T.astype(f16)                                 # [D, S]
    xPk = np.ascontiguousarray(
        xT16.reshape(NKQ, KH, P, SC, 512)
        .transpose(2, 3, 0, 1, 4).reshape(P, -1))
    # wqP[p, (m, ko, col)] = wqT[ko*128+p, m*128+col]
    wqT16 = qrows.T.astype(f16)                               # [D, NQ]
    wqPk = np.ascontiguousarray(
        wqT16.reshape(KO, P, 4, P).transpose(1, 2, 0, 3).reshape(P, -1))
    wkPk = np.ascontiguousarray(
        krows.T.astype(f16).reshape(KO, P, P)
        .transpose(1, 0, 2).reshape(P, -1))
    wvPk = np.ascontiguousarray(
        vrows.T.astype(f16).reshape(KO, P, P)
        .transpose(1, 0, 2).reshape(P, -1))
    return {
        "xP": xPk,
        "wqP": wqPk,
        "wkP": wkPk,
        "wvP": wvPk,
        "woT": np.ascontiguousarray(wocols.T).astype(f16),
        "ck": ctab.astype(f16),
        "sk": stab.astype(f16),
        "maskT": maskT,
    }


_CACHED_NC = None


def _get_nc():
    global _CACHED_NC
    if _CACHED_NC is None:
        _CACHED_NC = _build_bass()
    return _CACHED_NC


def _make_in_maps(x, wq, wk, wv, wo, attention_mask):
    tables = _rope_tables()
    return [
        _prep_core_inputs(x, wq, wk, wv, wo, attention_mask, c, tables)
        for c in range(N_CORES)
    ]


def kernel(x, wq, wk, wv, wo, attention_mask):
    from concourse.bass_utils import run_bass_kernel_spmd

    x = np.asarray(x, dtype=np.float32)
    wq = np.asarray(wq, dtype=np.float32)
    wk = np.asarray(wk, dtype=np.float32)
    wv = np.asarray(wv, dtype=np.float32)
    wo = np.asarray(wo, dtype=np.float32)
    attention_mask = np.asarray(attention_mask)

    nc = _get_nc()
    in_maps = _make_in_maps(x, wq, wk, wv, wo, attention_mask)
    res = run_bass_kernel_spmd(nc, in_maps, core_ids=list(range(N_CORES)))
    out = np.zeros((B, S, D), dtype=np.float32)
    for c in range(N_CORES):
        out[c // 4] += res.results[c]["part"].astype(np.float32)
    return out


if __name__ == "__main__":
    rng = np.random.default_rng(0)
    ins = {
        "x": rng.standard_normal((B, S, D), dtype=np.float32),
        "wq": (rng.standard_normal((H * HD, D)) * 0.02).astype(np.float32),
        "wk": (rng.standard_normal((KVH * HD, D)) * 0.02).astype(np.float32),
        "wv": (rng.standard_normal((KVH * HD, D)) * 0.02).astype(np.float32),
        "wo": (rng.standard_normal((D, H * HD)) * 0.02).astype(np.float32),
        "attention_mask": np.ones((B, S), dtype=np.int32),
    }
    out = kernel(**ins)
    print("kernel ran, out shape", out.shape, "std", out.std())
